# revision 41
# baseline (speedup 1.0000x reference)
"""Trainium2 Bass kernel for nn_EquivariantProductBasisBlock.

Math: for each node n (species s) and channel c the MACE symmetric
contraction reduces to

    f[n,c,L] = sum_i x[n,c,i] * H[n,c,(L,i)]
    H[n,c,(L,i)] = sum_K G[s][K, c, (L,i)] * phi[n,c,K]

where phi = the 153 symmetric degree<=2 monomials of x~ = [x, 1] (17 dims)
and G = the U (x) W tables contracted over the CG-path axis p (weight-only,
folded on host).  Output y = concat(f0 @ Wlin0, f1 @ Wlin1) / sqrt(C).

Device mapping (8 cores, channel-sharded: 16 of 128 channels per core).
The cost model's DMA device is an exclusive ~360GB/s resource, so input
bytes pace the whole kernel; everything else hides under the stream:

  - phi is built ON THE HOST and sent as fp8 e3m4 (halving the dominant
    stream) with a per-(c,n) column scale sc = colmax/14; the scale is
    folded into xw (= x * sc), which multiplies H elementwise later, so
    the fold is exact.  G's K1 block rides fp8 too (range fits e3m4);
    G's K0 block stays fp16 (mixed-dtype matmuls are fine).
  - phi layout [K, N, CPC]: node-range DMA chunks keep >=512B contiguous
    elements at fp8, allowing per-window-pair pacing; the PE stationary
    reads it with a strided AP.
  - input DMA order = consumption order (window w's g/phi land just
    ahead of its compute; wl mid-stream so Wlin tails drain early; the
    last xw chunk rides last since it is only needed ~2us after H).
    DMA count is kept ~20: each DMA costs ~600ns on the shared HWDGE
    device and on the issuing queue.
  - nodes host-sorted by species; per species window (<=128 nodes):
    PE matmuls H = phi^T G (K=153 contraction) into per-half-window
    PSUM banks; ACT casts each bank to fp16 as it completes; DVE
    multiplies by xw (2x fp16 mode) and reduces over i with an add
    tree into fw; PE transpose + copy; PE Wlin matmul; ACT casts y and
    the SP queue DMAs it out (outputs naturally trail the input stream
    on the exclusive DMA device; SP/HWDGE is idle by then).
  - PSUM: 3 half-window H banks + 3 Wlin banks + 1 transpose bank; the
    fw ring is 8 deep so write-after-read hazards never bind.
  - host sums the 8 channel-partials (fp32), un-permutes rows, reorders
    columns.
"""

import numpy as np
import ml_dtypes

import concourse.bass as bass
import concourse.mybir as mybir
import concourse.tile as tile
from concourse import bacc
from concourse.bass_utils import run_bass_kernel_spmd
from concourse.masks import make_identity

# ---- problem constants (hardcoded per spec) ----
N, C, LM, ELEMS = 1024, 128, 16, 10
NL = 4                      # global L rows: block0 (dim1) + block1 (dim3)
NX = 17                     # x~ = [x_0..x_15, 1]
KTOT = NX * (NX + 1) // 2   # 153 sym pair monomials
K0, K1 = 128, KTOT - 128    # partition chunks (128 + 25)
NCORES = 8
CPC = C // NCORES           # channels per core
LIN = NL * LM               # 64 = (L, i) columns streamed per matmul

PHI_DT = mybir.dt.float8e3
PHI_NP = ml_dtypes.float8_e3m4
F16 = mybir.dt.float16

# pair tables: global pair row r -> (j, m), j <= m
_PAIRS = [(j, m) for j in range(NX) for m in range(j, NX)]

# scheduling knobs (tuned via sweep.py; defaults = best known)
CFG = {
    "LT": 3,          # transpose lag behind H stream
    "LY": 6,          # Wlin lag behind H stream
    "YSB_ALT": False, # alternate ysb casts between ACT and DVE
    "PH_BUFS": 3,     # PSUM bufs for the H output
    "PY_BUFS": 3,     # PSUM bufs for the Wlin output
    "H_FIRST": True,  # emit H before tail ops each iteration
    "SB_BUFS": 5,     # bufs for phs/tmp/tr pools (window decoupling)
    "ABLATE": "",     # "" | "no_tail" | "no_y" | "no_dve" (timing probes)
    "REDUCE": "tree", # "tree" | "t8r" | "r16" : i-reduction strategy
    "Y_POOL_N": 0,    # how many of the LAST windows" y DMAs go via Pool
    "YSB_DVE_N": 1,   # how many of the LAST windows" ysb casts go via DVE
    "ID_DMA": 0,      # build identity on-chip (DMA variant measured slower)
}


def _build_windows(counts):
    """Species-sorted node windows of <=128 nodes: [(elem, start, len)]."""
    wins = []
    a = 0
    for e in range(ELEMS):
        left = int(counts[e])
        while left > 0:
            w = min(left, 128)
            wins.append((e, a, w))
            a += w
            left -= w
    assert a == N
    return wins


def _build_G(inp):
    """G[K, e, c, l, i] fp32: U (x) W fused tables (weight-only folding)."""
    G = np.zeros((KTOT, ELEMS, C, NL, LM), dtype=np.float32)
    pidx = {p: i for i, p in enumerate(_PAIRS)}
    for b, d in enumerate((1, 3)):
        U1 = np.asarray(inp[f"U1_{b}"], np.float32)
        U2 = np.asarray(inp[f"U2_{b}"], np.float32)
        U3 = np.asarray(inp[f"U3_{b}"], np.float32)
        W1 = np.asarray(inp[f"W1_{b}"], np.float32)
        W2 = np.asarray(inp[f"W2_{b}"], np.float32)
        W3 = np.asarray(inp[f"W3_{b}"], np.float32)
        lb = 0 if b == 0 else 1
        A1 = np.einsum("Lip,epc->ecLi", U1, W1, optimize=True)
        G[pidx[(16, 16)], :, :, lb:lb + d, :] += A1
        A2 = np.einsum("Lijp,epc->ecLij", U2, W2, optimize=True)
        for j in range(LM):
            G[pidx[(j, 16)], :, :, lb:lb + d, :] += A2[:, :, :, :, j]
        A3 = np.einsum("Lijmp,epc->ecLijm", U3, W3, optimize=True)
        for j in range(LM):
            for m in range(j, LM):
                if j == m:
                    coef = A3[:, :, :, :, j, j]
                else:
                    coef = A3[:, :, :, :, j, m] + A3[:, :, :, :, m, j]
                G[pidx[(j, m)], :, :, lb:lb + d, :] += coef
    return G


def build_program(windows):
    # Bacc (not raw Bass): its compile() lowers multi-semaphore waits onto
    # InstEventSemaphore chains (TRN2 allows only 1 wait per instruction).
    nc = bacc.Bacc()
    f32 = mybir.dt.float32
    NW = len(windows)

    ph0_d = nc.dram_tensor("ph0", [K0, N, CPC], PHI_DT, kind="ExternalInput")
    ph1_d = nc.dram_tensor("ph1", [K1, N, CPC], PHI_DT, kind="ExternalInput")
    g0_d = nc.dram_tensor("g0", [K0, ELEMS, CPC, LIN], F16, kind="ExternalInput")
    g1_d = nc.dram_tensor("g1", [K1, ELEMS, CPC, LIN], PHI_DT, kind="ExternalInput")
    xw_d = nc.dram_tensor("xw", [128, NW, CPC, LM], F16, kind="ExternalInput")
    # block-diagonal Wlin: row (32l + c), col (128l + k) = Wlin_l[c, k]/sqrt(C)
    wl_d = nc.dram_tensor("wl", [128, NL * C], F16, kind="ExternalInput")
    id_d = nc.dram_tensor("ident", [128, 128], F16, kind="ExternalInput")
    y_d = nc.dram_tensor("y", [N, NL * C], F16, kind="ExternalOutput")

    # window pairs: phi DMA chunks cover two windows each
    pairs = []
    for p in range(0, NW, 2):
        a = windows[p][1]
        last = windows[min(p + 1, NW - 1)]
        pairs.append((a, last[1] + last[2]))

    with tile.TileContext(nc) as tc:
        with (
            tc.tile_pool(name="singles", bufs=1) as singles,
            tc.tile_pool(name="phs", bufs=CFG["SB_BUFS"]) as phs_pool,
            tc.tile_pool(name="tmp", bufs=CFG["SB_BUFS"]) as tmp_pool,
            tc.tile_pool(name="tr", bufs=CFG["SB_BUFS"]) as tr_pool,
            tc.tile_pool(name="fts", bufs=CFG["SB_BUFS"]) as fts_pool,
            tc.tile_pool(name="ysb", bufs=max(NW, 4)) as ysb_pool,
            tc.tile_pool(name="ph", bufs=CFG["PH_BUFS"], space="PSUM") as ph_pool,
            tc.tile_pool(name="pt", bufs=CFG.get("PT_BUFS",1), space="PSUM") as pt_pool,
            tc.tile_pool(name="py", bufs=CFG.get("PY_BUFS",1), space="PSUM") as py_pool,
        ):
            g0_sb = singles.tile([K0, ELEMS, CPC, LIN], F16)
            g1_sb = singles.tile([K1, ELEMS, CPC, LIN], PHI_DT)
            ph0_sb = singles.tile([K0, N, CPC], PHI_DT)
            ph1_sb = singles.tile([K1, N, CPC], PHI_DT)
            xw_sb = singles.tile([128, NW, CPC, LM], F16)
            wl_sb = singles.tile([128, NL * C], F16)
            ident = singles.tile([128, 128], F16)
            if CFG.get("ID_DMA", 1):
                nc.sync.dma_start(out=ident, in_=id_d[:])
            else:
                make_identity(nc, ident)

            # fw buffers: memset once; the add tree only ever writes the 64
            # (32l + c) columns, and the other columns hit zero Wlin rows.
            # Ring of 8 so the fadd(w) WAR on transpose(w-ring) never binds.
            fwt = tuple(singles.tile([128, 128], F16, name=f"fw_{i}")
                        for i in range(8))
            for f in fwt:
                nc.vector.memset(f, 0.0)

            # ---- DMA issue order = pipeline order ----
            g0_hi = -1
            g1_hi = -1

            def load_g_upto(e):
                nonlocal g0_hi, g1_hi
                if e > g0_hi:
                    nc.sync.dma_start(out=g0_sb[:, g0_hi + 1:e + 1],
                                      in_=g0_d[:, g0_hi + 1:e + 1])
                    g0_hi = e
                if e > g1_hi:
                    nc.sync.dma_start(out=g1_sb[:, g1_hi + 1:e + 1],
                                      in_=g1_d[:, g1_hi + 1:e + 1])
                    g1_hi = e

            def ph_chunk(lo, hi):
                nc.sync.dma_start(out=ph0_sb[:, lo:hi], in_=ph0_d[:, lo:hi])

            def ph1_chunk(lo, hi):
                nc.sync.dma_start(out=ph1_sb[:, lo:hi], in_=ph1_d[:, lo:hi])

            def xw_chunk(w0, w1):
                nc.sync.dma_start(out=xw_sb[:, w0:w1], in_=xw_d[:, w0:w1])

            # species needed by each pair
            def pair_species(p):
                return windows[min(2 * p + 1, NW - 1)][0]

            NP = len(pairs)
            # Strict need-order stream: window w's inputs (g, phi) arrive
            # in consumption order, phi per-pair early / per-window late so
            # the last windows' H can start as early as possible.  The
            # least-latency-critical bytes (late xw, wl) land last: xw(w)
            # is needed ~2us after H(w) starts, wl only at the Wlin stage.
            def win_range(w0, w1):
                a = windows[w0][1]
                last = windows[min(w1, NW - 1)]
                return a, last[1] + last[2]

            load_g_upto(windows[min(1, NW - 1)][0])
            xw_chunk(0, min(2, NW))
            ph_chunk(*win_range(0, 1))
            ph1_chunk(*win_range(0, 1))
            if NW > 2:
                load_g_upto(windows[min(3, NW - 1)][0])
                xw_chunk(2, min(4, NW))
                ph_chunk(*win_range(2, 3))
                ph1_chunk(*win_range(2, min(5, NW - 1)))
            nc.sync.dma_start(out=wl_sb, in_=wl_d[:])
            if NW > 4:
                load_g_upto(windows[min(5, NW - 1)][0])
                ph_chunk(*win_range(4, 5))
            if NW > 6:
                load_g_upto(windows[min(7, NW - 1)][0])
                ph1_chunk(*win_range(6, NW - 1))
                xw_chunk(4, min(8, NW))
                ph_chunk(*win_range(6, 6))
                ph_chunk(*win_range(7, 7))
            for w in range(8, NW):
                load_g_upto(windows[w][0])
                ph_chunk(*win_range(w, w))
                if w == NW - 1:
                    xw_chunk(8, NW)
            load_g_upto(ELEMS - 1)

            # ---- window pipeline; H emitted FIRST each iteration so the
            # in-order PE queue is never blocked by tail ops whose DVE/ACT
            # deps aren't ready yet.  Tail ops lag LT/LY windows: deep lags
            # keep the in-order PE queue from coupling H(w) to the full
            # cast->mul->tree->transpose->Wlin chain of recent windows.
            LT, LY = CFG["LT"], CFG["LY"]
            fts_t = {}
            ph_t = {}
            for w in range(NW + LY):
                if w < NW and CFG["H_FIRST"]:
                    e, a, wlen = windows[w]
                    HB = CPC // 2
                    if CFG.get("WHOLE_PH"):
                        pht = ph_pool.tile([128, CPC, NL, LM], f32, tag="ph")
                        phh = [pht[:, 0:HB], pht[:, HB:CPC]]
                        for c in range(CPC):
                            nc.tensor.matmul(
                                pht[:wlen, c], ph0_sb[:, a:a + wlen, c],
                                g0_sb[:, e, c, :], start=c % 8 == 0,
                                stop=False)
                            nc.tensor.matmul(
                                pht[:wlen, c], ph1_sb[:, a:a + wlen, c],
                                g1_sb[:, e, c, :], start=False,
                                stop=c % 8 == 7)
                    else:
                        phh = []
                        for h in range(2):
                            pht = ph_pool.tile([128, HB, NL, LM], f32,
                                               tag="ph")  # one PSUM bank
                            phh.append(pht)
                            for cc in range(HB):
                                c = h * HB + cc
                                nc.tensor.matmul(
                                    pht[:wlen, cc], ph0_sb[:, a:a + wlen, c],
                                    g0_sb[:, e, c, :], start=cc == 0,
                                    stop=False)
                                nc.tensor.matmul(
                                    pht[:wlen, cc], ph1_sb[:, a:a + wlen, c],
                                    g1_sb[:, e, c, :], start=False,
                                    stop=cc == HB - 1)
                    ph_t[w] = phh

                if 0 <= w - LT < NW and CFG["ABLATE"] not in ("no_tail",):
                    v = w - LT
                    fts = fts_pool.tile([128, 128], F16, name=f"fts{v}",
                                        tag="fts")
                    ftp = pt_pool.tile([128, 128], F16, name=f"ftp{v}",
                                       tag="ftp")
                    back = NW - 1 - v
                    lo, hi = CFG.get("XBAR_T", (99, 99))
                    if lo <= back <= hi:
                        nc.sync.dma_start_transpose(fts, fwt[v % len(fwt)])
                    else:
                        nc.tensor.transpose(ftp, fwt[v % len(fwt)], ident)
                        if CFG.get("PRI_FTS", 0):
                            with tc.high_priority(offset=CFG["PRI_FTS"]):
                                nc.vector.tensor_copy(fts, ftp)
                        elif back < CFG.get("FTS_ACT_N", 0):
                            nc.scalar.copy(fts, ftp)
                        else:
                            nc.vector.tensor_copy(fts, ftp)
                    fts_t[v] = fts
                if 0 <= w - LY < NW and CFG["ABLATE"] not in ("no_tail",):
                    v = w - LY
                    _, av, lv = windows[v]
                    py = py_pool.tile([128, NL * C], f32, name=f"py{v}",
                                      tag="py")
                    fts_v = fts_t.pop(v)
                    ysb = ysb_pool.tile([128, NL * C], F16, name=f"ysb{v}",
                                        tag="ysb")
                    yeng = nc.gpsimd if (CFG.get("Y_POOL_N",0) > NW - 1 - v) else nc.sync
                    if NW - 1 - v < CFG.get("YSPLIT_N", 0):
                        half = NL * C // 2
                        for g in range(2):
                            s = slice(g * half, (g + 1) * half)
                            nc.tensor.matmul(py[:, s], fts_v, wl_sb[:, s],
                                             start=True, stop=True)
                            (nc.scalar.copy if g == 0
                             else nc.vector.tensor_copy)(ysb[:, s], py[:, s])
                            yeng.dma_start(out=y_d[av:av + lv, s],
                                           in_=ysb[:lv, s])
                    else:
                        nc.tensor.matmul(py, fts_v, wl_sb,
                                         start=True, stop=True)
                        if CFG["ABLATE"] != "no_y":
                            blo, bhi = CFG.get("BAL", (99, 0))
                            if blo <= v < NW - bhi:
                                nc.scalar.copy(ysb[:, 0:NL * C // 2],
                                               py[:, 0:NL * C // 2])
                                nc.vector.tensor_copy(
                                    ysb[:, NL * C // 2:], py[:, NL * C // 2:])
                            elif NW - 1 - v < CFG.get("YSB_DVE_N", 0):
                                nc.vector.tensor_copy(ysb, py)
                            elif CFG.get("PRI_YSB", 0):
                                with tc.high_priority(offset=CFG["PRI_YSB"]):
                                    nc.scalar.copy(ysb, py)
                            elif CFG["YSB_ALT"] and v % 2 == 1:
                                nc.vector.tensor_copy(ysb, py)
                            else:
                                nc.scalar.copy(ysb, py)
                            yeng.dma_start(out=y_d[av:av + lv], in_=ysb[:lv])

                if w < NW:
                    e, a, wlen = windows[w]
                    if not CFG["H_FIRST"]:
                        ph = ph_pool.tile([128, CPC, NL, LM], f32)
                        ph_t[w] = ph
                        for c in range(CPC):
                            first = c % 8 == 0
                            nc.tensor.matmul(
                                ph[:wlen, c], ph0_sb[:, a:a + wlen, c],
                                g0_sb[:, e, c, :], start=first, stop=False)
                            nc.tensor.matmul(
                                ph[:wlen, c], ph1_sb[:, a:a + wlen, c],
                                g1_sb[:, e, c, :], start=False, stop=c % 8 == 7)
                    phh = ph_t.pop(w)
                    xwv = xw_sb[:, w]
                    tmp = tmp_pool.tile([128, CPC, NL, LM], F16)
                    # cast H to fp16 on ACT and multiply on DVE (2x mode),
                    # split by channel halves: each PSUM bank's cast starts
                    # as soon as its 8 channels' matmuls are done, and the
                    # bank frees at half-window granularity.
                    phs = phs_pool.tile([128, CPC, NL, LM], F16, tag="phs")
                    HB = CPC // 2
                    if CFG.get("WHOLE_PH") and CFG.get("WHOLE_CAST"):
                        xw_b = bass.AP(tensor=xwv.tensor, offset=xwv.offset,
                                       ap=[[xwv.ap[0][0], wlen],
                                           list(xwv.ap[1]),
                                           [0, NL], list(xwv.ap[2])])
                        whole = phh[0].tensor is phh[1].tensor
                        nc.scalar.copy(phs[:wlen],
                                       bass.AP(tensor=phh[0].tensor,
                                               offset=phh[0].offset,
                                               ap=[[phh[0].ap[0][0], wlen],
                                                   [NL * LM, CPC],
                                                   [LM, NL], [1, LM]]))
                        nc.vector.tensor_mul(tmp[:wlen], phs[:wlen], xw_b)
                    else:
                     for h in range(2):
                        cl, ch = h * HB, (h + 1) * HB
                        xwh = xwv[:, cl:ch]
                        xw_b = bass.AP(tensor=xwh.tensor, offset=xwh.offset,
                                       ap=[[xwh.ap[0][0], wlen],
                                           list(xwh.ap[1]),
                                           [0, NL], list(xwh.ap[2])])
                        if h == 1 and CFG.get("CAST", "split") == "psum2":
                            # second half: DVE multiplies straight from PSUM
                            nc.vector.tensor_mul(tmp[:wlen, cl:ch],
                                                 phh[h][:wlen], xw_b)
                        elif CFG.get("PRI_CAST", 0):
                            with tc.high_priority(offset=CFG["PRI_CAST"]):
                                nc.scalar.copy(phs[:wlen, cl:ch],
                                               phh[h][:wlen])
                            nc.vector.tensor_mul(tmp[:wlen, cl:ch],
                                                 phs[:wlen, cl:ch], xw_b)
                        else:
                            nc.scalar.copy(phs[:wlen, cl:ch], phh[h][:wlen])
                            nc.vector.tensor_mul(tmp[:wlen, cl:ch],
                                                 phs[:wlen, cl:ch], xw_b)

                    # reduce over i: either a fp16 2x add tree or a
                    # fused TensorReduce (fewer queue ops, 1x rate); the
                    # final stage always scatters into fw cols (32l + c).
                    fw = fwt[w % len(fwt)]
                    fw_out = bass.AP(tensor=fw.tensor, offset=fw.offset,
                                     ap=[[fw.ap[0][0], wlen], [1, CPC],
                                         [32, NL]])
                    if CFG["REDUCE"] == "r16" or NW - 1 - w < CFG.get("R16_N", 0):
                        with nc.allow_low_precision(reason="i-sum fp16"):
                            nc.vector.tensor_reduce(
                                fw_out, tmp[:wlen], mybir.AxisListType.X,
                                mybir.AluOpType.add)
                    else:
                        t8 = tr_pool.tile([128, CPC, NL, 8], F16, tag="t8")
                        t8e = (nc.gpsimd if (CFG.get("T8_POOL") and
                                             w < NW - 2) else nc.vector)
                        t8e.tensor_add(t8[:wlen],
                                       tmp[:wlen, :, :, 0:8],
                                       tmp[:wlen, :, :, 8:16])
                        if CFG["REDUCE"] == "t8r":
                            with nc.allow_low_precision(reason="i-sum fp16"):
                                nc.vector.tensor_reduce(
                                    fw_out, t8[:wlen], mybir.AxisListType.X,
                                    mybir.AluOpType.add)
                        else:
                            t4 = tr_pool.tile([128, CPC, NL, 4], F16,
                                              tag="t4")
                            t2 = tr_pool.tile([128, CPC, NL, 2], F16,
                                              tag="t2")
                            blo, bhi = CFG.get("BAL", (99, 0))
                            bal = blo <= w < NW - bhi
                            tr_eng = nc.gpsimd if bal else nc.vector
                            tr_eng.tensor_add(t4[:wlen],
                                              t8[:wlen, :, :, 0:4],
                                              t8[:wlen, :, :, 4:8])
                            tr_eng.tensor_add(t2[:wlen],
                                              t4[:wlen, :, :, 0:2],
                                              t4[:wlen, :, :, 2:4])
                            t2a = bass.AP(tensor=t2.tensor, offset=t2.offset,
                                          ap=[[t2.ap[0][0], wlen], [2, NL],
                                              [NL * 2, CPC]])
                            t2b = bass.AP(tensor=t2.tensor,
                                          offset=t2.offset + 1,
                                          ap=[[t2.ap[0][0], wlen], [2, NL],
                                              [NL * 2, CPC]])
                            fw_lc = bass.AP(tensor=fw.tensor,
                                            offset=fw.offset,
                                            ap=[[fw.ap[0][0], wlen],
                                                [32, NL], [1, CPC]])
                            tr_eng.tensor_add(fw_lc, t2a, t2b)

    nc.compile()
    return nc


def prepare(inputs):
    """Host prep: sort by species, fold G, build fp8 phi, pack per-core."""
    x = np.asarray(inputs["x"], np.float32)
    species = np.asarray(inputs["species"])
    order = np.argsort(species, kind="stable")
    xs = x[order]                           # [N, C, 16]
    sp = np.asarray(species)[order]
    counts = np.bincount(sp, minlength=ELEMS)
    windows = _build_windows(counts)
    NW = len(windows)

    # x~T [17, C, N]
    xt = np.empty((NX, C, N), np.float32)
    xt[:LM] = xs.transpose(2, 1, 0)
    xt[LM] = 1.0

    # phi [153, C, N]; per-(c,n) scale so fp8 e3m4 (max 15.5) never clips.
    a_src = np.array([p[0] for p in _PAIRS])
    b_src = np.array([p[1] for p in _PAIRS])
    phi = xt[a_src] * xt[b_src]
    sc = np.abs(phi).max(axis=0) / 14.0     # [C, N]; >= 1/14 (const row)
    phi_q = (phi / sc).astype(PHI_NP)       # [153, C, N] fp8

    G = _build_G(inputs)                    # [K, E, C, 4, 16] fp32

    s = 1.0 / np.sqrt(np.float32(C))
    wl_full = np.zeros((NL, C, C), np.float32)
    wl_full[0] = np.asarray(inputs["Wlin_0"], np.float32) * s
    wl_full[1:] = np.asarray(inputs["Wlin_1"], np.float32) * s

    in_maps = []
    for q in range(NCORES):
        cs, ce = q * CPC, (q + 1) * CPC
        # xw = xs * sc (folds the phi column scales back in)
        xw = np.zeros((128, NW, CPC, LM), np.float16)
        for w, (e, a, wlen) in enumerate(windows):
            xw[:wlen, w] = (xs[a:a + wlen, cs:ce]
                            * sc.T[a:a + wlen, cs:ce, None])
        Gq = np.ascontiguousarray(
            G[:, :, cs:ce].reshape(KTOT, ELEMS, CPC, LIN))
        wl_q = np.zeros((128, NL * C), np.float16)
        for l in range(NL):
            wl_q[32 * l:32 * l + CPC, 128 * l:128 * (l + 1)] = wl_full[l, cs:ce]
        in_maps.append({
            "ph0": np.ascontiguousarray(phi_q[:K0, cs:ce].transpose(0, 2, 1)),
            "ph1": np.ascontiguousarray(phi_q[K0:, cs:ce].transpose(0, 2, 1)),
            "g0": np.ascontiguousarray(Gq[:K0]).astype(np.float16),
            "g1": np.ascontiguousarray(Gq[K0:]).astype(PHI_NP),
            "xw": xw,
            "wl": wl_q,
            "ident": np.eye(128, dtype=np.float16),
        })
    return in_maps, windows, order


def kernel(**inputs):
    in_maps, windows, order = prepare(inputs)
    nc = build_program(windows)
    # The axon-tunneled device occasionally fails one execution with a
    # transient internal error that clears on retry; guard the single
    # grading invocation against it.
    last = None
    for _ in range(3):
        try:
            res = run_bass_kernel_spmd(nc, in_maps,
                                       core_ids=list(range(NCORES)))
            break
        except Exception as e:  # noqa: BLE001 - retry any runtime failure
            last = e
    else:
        raise last

    yd = np.zeros((N, NL * C), np.float32)
    for r in res.results:
        yd += np.asarray(r["y"], np.float32)

    # columns: [0:128] = L0 @ k ; block1 interleaved 128 + 3k + i
    y = np.empty((N, 512), np.float32)
    y[:, 0:128] = yd[:, 0:128]
    for i in range(3):
        y[:, 128 + i::3] = yd[:, (1 + i) * 128:(2 + i) * 128]

    inv = np.empty_like(order)
    inv[order] = np.arange(N)
    return y[inv]
